# revision 24
# baseline (speedup 1.0000x reference)
"""Trainium2 Bass kernel for multi-head attention (B=4, N=2048, C=1024, H=16).

Sharding (8 cores, no collectives): core c handles batch b = c//2 and
sequence-half h2 = c%2 (q rows [h2*1024, h2*1024+1024)). Each core computes
k/v for the full sequence of its batch (duplicated within the pair), its
q-half, attention for all 16 heads, and the output projection for its rows.
Host concatenates the 8 row-blocks. Host rotates x columns so the local
q-half is always cols [0, 1024) (k-order permutation is softmax-invariant).

Schedule (v1 restructure): the scalar engine (exp) is the intrinsic
bottleneck (~265us busy); the program is ordered so it starts at ~10us and
never starves: K/Q projection for head-pair 0 runs first, V projection and
the next pair's K/Q matmuls are interleaved as fillers into each pair's
scores/exp/ctx pipeline, and attn@V uses N=1024 fp16 moving operands.
"""
import sys

sys.path.insert(0, "/opt/trn_rl_repo")

import numpy as np

B, N, C = 4, 2048, 1024
H = 16
D = C // H
SCALE = np.float32(1.0) / np.sqrt(D).astype(np.float32)
NCORES = 8
NLOC = N // 2            # q rows per core
NKC = N // 128           # 16 k-chunks
NPAIR = H // 2           # 8 head pairs
VW = 65                  # v columns + ones
_cache = {}


def _build():
    import concourse.bacc as bacc
    import concourse.tile as tile
    import concourse.mybir as mybir

    f32 = mybir.dt.float32
    f16 = mybir.dt.bfloat16

    nc = bacc.Bacc("TRN2", target_bir_lowering=False, debug=False,
                   num_devices=NCORES)

    xT_d = nc.dram_tensor("xT", [128, 8 * N], f16, kind="ExternalInput").ap()
    wq_d = nc.dram_tensor("wq", [128, 8 * C], f16, kind="ExternalInput").ap()
    wk_d = nc.dram_tensor("wk", [128, 8 * C], f16, kind="ExternalInput").ap()
    wv_d = nc.dram_tensor("wv", [128, 8 * C], f16, kind="ExternalInput").ap()
    wo_d = nc.dram_tensor("wo", [128, 8 * C], f16, kind="ExternalInput").ap()
    bo_d = nc.dram_tensor("bo_b", [128, C], f32, kind="ExternalInput").ap()
    out_d = nc.dram_tensor("out", [NLOC, C], f32, kind="ExternalOutput").ap()

    with tile.TileContext(nc) as tc:
        with tc.tile_pool(name="sc_ps", bufs=2, space="PSUM") as sc_ps, \
             tc.tile_pool(name="mm_ps", bufs=2, space="PSUM") as mm_ps, \
             tc.tile_pool(name="ctx_ps", bufs=1, space="PSUM") as ctx_ps, \
             tc.tile_pool(name="big", bufs=1) as big, \
             tc.tile_pool(name="wbig", bufs=1) as wbig, \
             tc.tile_pool(name="w_pool", bufs=2) as w_pool, \
             tc.tile_pool(name="kt_pool", bufs=2) as kt_pool, \
             tc.tile_pool(name="qt_pool", bufs=2) as qt_pool, \
             tc.tile_pool(name="pt_pool", bufs=16) as pt_pool, \
             tc.tile_pool(name="nrm_pool", bufs=1) as nrm_pool, \
             tc.tile_pool(name="out_pool", bufs=2) as out_pool:
            # resident tensors
            XT = big.tile([128, 8 * N], f16, name="XT")
            VA = big.tile([128, H * NKC * VW], f16, name="VA")
            nc.vector.memset(
                VA[:].rearrange("p (hk w) -> p hk w", w=VW)[:, :, 64], 1.0)
            ctxT = big.tile([128, NPAIR * NLOC], f16, name="ctxT")

            dmaq = [nc.sync, nc.gpsimd, nc.scalar]

            # ---- DMA schedule ----
            # wk/wq for hp0 first, then XT local-half (p1), WV, XT far-half
            # (p2). Rotating queues; per-queue FIFO approximates priority.
            wkq_tiles = {}

            def dma_wkq(hp):
                wk_t = w_pool.tile([128, 8 * 128], f16, name="wkp")
                nc.sync.dma_start(
                    out=wk_t[:], in_=wk_d[:, hp * 1024:(hp + 1) * 1024])
                wq_t = w_pool.tile([128, 8 * 128], f16, name="wqp")
                nc.sync.dma_start(
                    out=wq_t[:], in_=wq_d[:, hp * 1024:(hp + 1) * 1024])
                wkq_tiles[hp] = (wk_t, wq_t)

            dma_wkq(0)
            for half in range(2):          # p1 = local q-half cols of each cc
                for cc in range(8):
                    q = dmaq[(half * 8 + cc) % 3]
                    q.dma_start(
                        out=XT[:, cc * N + half * 512:cc * N + half * 512 + 512],
                        in_=xT_d[:, cc * N + half * 512:cc * N + half * 512 + 512])
            WV = wbig.tile([128, 8 * C], f16, name="wband")
            for cc in range(8):
                dmaq[cc % 3].dma_start(out=WV[:, cc * C:(cc + 1) * C],
                                       in_=wv_d[:, cc * C:(cc + 1) * C])
            for half in range(2):          # p2 = far-half cols
                for cc in range(8):
                    q = dmaq[(half * 8 + cc) % 3]
                    q.dma_start(
                        out=XT[:, cc * N + 1024 + half * 512:
                               cc * N + 1024 + half * 512 + 512],
                        in_=xT_d[:, cc * N + 1024 + half * 512:
                                 cc * N + 1024 + half * 512 + 512])
            dma_wkq(1)
            BO = big.tile([128, C], f32, name="BO")
            nc.sync.dma_start(out=BO[:], in_=bo_d)

            # ---- filler machinery: one mm-group = 8 matmuls + DVE cast ----
            def kq_group(hp, which, g):
                wk_t, wq_t = wkq_tiles[hp]
                w, dst = (wk_t, kts[hp]) if which == "k" else (wq_t, qts[hp])
                ps = mm_ps.tile([128, 512], f32, name="ps")
                for cc in range(8):
                    nc.tensor.matmul(
                        ps[:], lhsT=w[:, cc * 128:(cc + 1) * 128],
                        rhs=XT[:, cc * N + g * 512:cc * N + (g + 1) * 512],
                        start=(cc == 0), stop=(cc == 7))
                nc.vector.tensor_copy(dst[:, g * 512:(g + 1) * 512], ps[:])

            def v_group(dh, nsub):
                ps = mm_ps.tile([128, 512], f32, name="ps")
                for cc in range(8):
                    nc.tensor.matmul(
                        ps[:],
                        lhsT=XT[:, cc * N + nsub * 128:cc * N + (nsub + 1) * 128],
                        rhs=WV[:, cc * C + dh * 512:cc * C + dh * 512 + 512],
                        start=(cc == 0), stop=(cc == 7))
                nc.vector.tensor_copy(
                    VA[:].rearrange("p (h c) -> p h c", h=H)
                       [:, dh * 8:(dh + 1) * 8, nsub * VW:nsub * VW + 64],
                    ps[:].rearrange("p (h d) -> p h d", h=8))

            kts, qts = {}, {}

            def alloc_kq(hp):
                kts[hp] = kt_pool.tile([128, N], f16, name="kT")
                qts[hp] = qt_pool.tile([128, NLOC], f16, name="qT")

            # ---- prologue: K/Q for hp0 ----
            alloc_kq(0)
            for g in range(2):
                kq_group(0, "q", g)
            for g in range(4):
                kq_group(0, "k", g)

            def ctx_mm(hp, hh, kc, pts, ctx_p):
                h = hp * 2 + hh
                for j in range(2):
                    nc.tensor.matmul(
                        ctx_p[:, j * 512:(j + 1) * 512],
                        lhsT=VA[:, (h * NKC + kc) * VW:(h * NKC + kc) * VW + 65],
                        rhs=pts[kc][:, j * 1024 + hh * 512:
                                    j * 1024 + hh * 512 + 512],
                        start=(kc == 0), stop=(kc == NKC - 1))

            def normalize(hp, hh, ctx_p):
                r0 = hh * 64
                den = nrm_pool.tile([1, NLOC], f32, name="den")
                nc.vector.tensor_copy(den[:], ctx_p[64:65, :])
                den_b = nrm_pool.tile([64, NLOC], f32, name="den_b")
                nc.gpsimd.partition_broadcast(den_b[:], den[:])
                rec = nrm_pool.tile([64, NLOC], f32, name="rec")
                nc.vector.reciprocal_approx_fast(out=rec[:], in_=den_b[:])
                nc.vector.tensor_tensor(
                    out=ctxT[r0:r0 + 64, hp * NLOC:(hp + 1) * NLOC],
                    in0=ctx_p[:64, :], in1=rec[:],
                    op=mybir.AluOpType.mult)

            # ---- main loop over head pairs ----
            for hp in range(NPAIR):
                if hp + 2 <= NPAIR - 1:
                    dma_wkq(hp + 2)
                if hp + 1 <= NPAIR - 1:
                    alloc_kq(hp + 1)
                # fillers placed inside the scores loop (one per kc slot)
                # and interleaved into the ctx(h1) loop.
                slot_fill = []
                tail_fill = []
                pre_ctx_fill = []
                if hp == 0:
                    # V dh0 needed by this hp's ctx; start at slot 4 so PE
                    # never blocks on the WV DMA while exp work is pending.
                    slot_fill = [None] * 4 + [
                        (lambda ns=ns: v_group(0, ns)) for ns in range(12)]
                    pre_ctx_fill = [(lambda ns=ns: v_group(0, ns))
                                    for ns in range(12, 16)]
                elif hp in (1, 2, 3):
                    lo = 6 * (hp - 1)
                    hi = min(16, lo + 6)
                    slot_fill = [(lambda ns=ns: v_group(1, ns))
                                 for ns in range(lo, hi)]
                if hp + 1 <= NPAIR - 1:
                    tail_fill = [(lambda g=g: kq_group(hp + 1, "k", g))
                                 for g in range(4)] + \
                        [(lambda g=g: kq_group(hp + 1, "q", g)) for g in range(2)]

                kT, qT = kts[hp], qts[hp]
                pts = []
                ctx_e = None
                for kc in range(NKC):
                    pt = pt_pool.tile([128, 2 * NLOC], f16, name="pt")
                    for j in range(2):
                        sp = sc_ps.tile([128, 1024], f32, name="sc")
                        for hh in range(2):
                            r0 = hh * 64
                            nc.tensor.matmul(
                                sp[:, hh * 512:(hh + 1) * 512],
                                lhsT=kT[r0:r0 + 64, kc * 128:(kc + 1) * 128],
                                rhs=qT[r0:r0 + 64, j * 512:(j + 1) * 512],
                                start=True, stop=True)
                        nc.scalar.activation(
                            pt[:, j * 1024:(j + 1) * 1024], sp[:],
                            mybir.ActivationFunctionType.Exp)
                    pts.append(pt)
                    if kc < len(slot_fill) and slot_fill[kc] is not None:
                        slot_fill[kc]()
                    if hp > 0:
                        # ctx for even head, lagging one kc behind exp
                        if kc == 0:
                            ctx_e = ctx_ps.tile([VW, NLOC], f32, name="ctx_p")
                        else:
                            ctx_mm(hp, 0, kc - 1, pts, ctx_e)
                for f in pre_ctx_fill:
                    f()
                if hp == 0:
                    ctx_e = ctx_ps.tile([VW, NLOC], f32, name="ctx_p")
                    for kc in range(1, NKC):
                        ctx_mm(hp, 0, kc - 1, pts, ctx_e)
                ctx_mm(hp, 0, NKC - 1, pts, ctx_e)
                normalize(hp, 0, ctx_e)
                ctx_o = ctx_ps.tile([VW, NLOC], f32, name="ctx_p")
                nf = 0
                for kc in range(NKC):
                    ctx_mm(hp, 1, kc, pts, ctx_o)
                    if kc % 3 == 2 and nf < len(tail_fill):
                        tail_fill[nf]()
                        nf += 1
                while nf < len(tail_fill):
                    tail_fill[nf]()
                    nf += 1
                normalize(hp, 1, ctx_o)

            # ---- projection ----
            WO = wbig.tile([128, 8 * C], f16, name="wband")
            for cc in range(8):
                dmaq[cc % 3].dma_start(out=WO[:, cc * C:(cc + 1) * C],
                                       in_=wo_d[:, cc * C:(cc + 1) * C])
            for nt in range(NLOC // 128):
                for ch in range(2):
                    ps = mm_ps.tile([128, 512], f32, name="ps")
                    for cc in range(8):
                        nc.tensor.matmul(
                            ps[:],
                            lhsT=ctxT[:, cc * NLOC + nt * 128:
                                      cc * NLOC + nt * 128 + 128],
                            rhs=WO[:, cc * C + ch * 512:cc * C + ch * 512 + 512],
                            start=(cc == 0), stop=(cc == 7))
                    ot = out_pool.tile([128, 512], f32)
                    nc.vector.tensor_tensor(
                        out=ot[:], in0=ps[:], in1=BO[:, ch * 512:(ch + 1) * 512],
                        op=mybir.AluOpType.add)
                    (nc.gpsimd, nc.sync)[(nt * 2 + ch) % 2].dma_start(
                        out=out_d[nt * 128:(nt + 1) * 128,
                                  ch * 512:(ch + 1) * 512],
                        in_=ot[:])

    nc.compile()
    return nc


def kernel(x, Wq, Wk, Wv, Wo, bo, _trace=False):
    from concourse.bass_utils import run_bass_kernel_spmd

    if "nc" not in _cache:
        _cache["nc"] = _build()
    nc = _cache["nc"]

    def _chunked(w):
        # [C, C] -> [128, 8*C]: row p holds w[cc*128+p, :] for cc = 0..7
        return np.ascontiguousarray(
            np.asarray(w, dtype=np.float32).astype(__import__("ml_dtypes").bfloat16)
            .reshape(8, 128, C).transpose(1, 0, 2).reshape(128, 8 * C))

    def _pair_chunked(w):
        # [C, C] -> [128, (hp, cc, 128)]: per head-pair contiguous blocks
        a = (np.asarray(w, dtype=np.float32).astype(__import__("ml_dtypes").bfloat16)
             .reshape(8, 128, 8, 128))           # [cc, p, hp, d]
        return np.ascontiguousarray(
            a.transpose(1, 2, 0, 3).reshape(128, 8 * C))

    x = np.asarray(x, dtype=np.float32)
    wq = _pair_chunked(np.asarray(Wq, dtype=np.float32) * SCALE)
    wk = _pair_chunked(Wk)
    wv = _chunked(Wv)
    wo = _chunked(Wo)
    bo_b = np.ascontiguousarray(
        np.broadcast_to(np.asarray(bo, dtype=np.float32), (128, C)))

    in_maps = []
    for c in range(NCORES):
        b, h2 = divmod(c, 2)
        xT = x[b].T.astype(__import__("ml_dtypes").bfloat16)
        # rotate so the local q-half is cols [0, NLOC); chunk to [128, 8*N]
        xT_rot = np.roll(xT, -h2 * NLOC, axis=1)
        xT_c = np.ascontiguousarray(
            xT_rot.reshape(8, 128, N).transpose(1, 0, 2).reshape(128, 8 * N))
        in_maps.append({"xT": xT_c, "wq": wq, "wk": wk, "wv": wv,
                        "wo": wo, "bo_b": bo_b})

    res = run_bass_kernel_spmd(nc, in_maps, core_ids=list(range(NCORES)),
                               trace=_trace, trace_cores=[0] if _trace else None)
    out = np.empty((B, N, C), dtype=np.float32)
    for c in range(NCORES):
        b, h2 = divmod(c, 2)
        out[b, h2 * NLOC:(h2 + 1) * NLOC, :] = res.results[c]["out"]
    if _trace:
        _cache["last_trace"] = res
    return out


# revision 26
# speedup vs baseline: 1.0140x; 1.0140x over previous
"""Trainium2 Bass kernel for multi-head attention (B=4, N=2048, C=1024, H=16).

Sharding (8 cores, no device collectives): core c handles batch b = c//2 and
head-group hg = c%2 (heads [8*hg, 8*hg+8), i.e. local head-pairs lp=0..3).
Each core computes q/k/v for its 8 heads over the FULL sequence, attention,
and a PARTIAL output projection (contraction over its 512 ctx channels,
all 2048 rows). The host sums the two partials of each batch and adds the
bias (exact fp32 reduction).

Schedule: the scalar engine (exp) is the intrinsic bottleneck (~265us busy).
The program keeps it fed: K/Q for pair 0 start immediately after the first
XT stripe lands, scores use 4-way tile_position packing (2 contraction
row-groups x 2 output col-groups -> 4 concurrent 512-col streams), V and the
next pair's K/Q matmuls fill the PE slack inside each block's scores/exp/ctx
pipeline. attn@V keeps the ones-column trick (VW=65) for free softmax
denominators.
"""
import sys

sys.path.insert(0, "/opt/trn_rl_repo")

import numpy as np

B, N, C = 4, 2048, 1024
H = 16
D = C // H
SCALE = np.float32(1.0) / np.sqrt(D).astype(np.float32)
NCORES = 8
NKC = N // 128           # 16 k-chunks
NLP = 4                  # local head pairs per core
VW = 65                  # v columns + ones
_cache = {}


def _build():
    import concourse.bacc as bacc
    import concourse.tile as tile
    import concourse.mybir as mybir

    f32 = mybir.dt.float32
    bf16 = mybir.dt.bfloat16

    nc = bacc.Bacc("TRN2", target_bir_lowering=False, debug=False,
                   num_devices=NCORES)

    xT_d = nc.dram_tensor("xT", [128, 8 * N], bf16, kind="ExternalInput").ap()
    wq_d = nc.dram_tensor("wq", [128, NLP * 1024], bf16,
                          kind="ExternalInput").ap()
    wk_d = nc.dram_tensor("wk", [128, NLP * 1024], bf16,
                          kind="ExternalInput").ap()
    wv_d = nc.dram_tensor("wv", [128, 8 * 512], bf16,
                          kind="ExternalInput").ap()
    wo_d = nc.dram_tensor("wo", [128, 4 * 1024], bf16,
                          kind="ExternalInput").ap()
    out_d = nc.dram_tensor("out", [N, C], f32, kind="ExternalOutput").ap()

    with tile.TileContext(nc) as tc:
        with tc.tile_pool(name="sc_ps", bufs=2, space="PSUM") as sc_ps, \
             tc.tile_pool(name="mm_ps", bufs=2, space="PSUM") as mm_ps, \
             tc.tile_pool(name="ctx_ps", bufs=1, space="PSUM") as ctx_ps, \
             tc.tile_pool(name="big", bufs=1) as big, \
             tc.tile_pool(name="wbig", bufs=1) as wbig, \
             tc.tile_pool(name="w_pool", bufs=2) as w_pool, \
             tc.tile_pool(name="kt_pool", bufs=2) as kt_pool, \
             tc.tile_pool(name="qt_pool", bufs=2) as qt_pool, \
             tc.tile_pool(name="pt_pool", bufs=16) as pt_pool, \
             tc.tile_pool(name="nrm_pool", bufs=1) as nrm_pool, \
             tc.tile_pool(name="out_pool", bufs=3) as out_pool:
            # resident tensors
            XT = big.tile([128, 8 * N], bf16, name="XT")
            VA = big.tile([128, 8 * NKC * VW], bf16, name="VA")
            nc.vector.memset(
                VA[:].rearrange("p (hk w) -> p hk w", w=VW)[:, :, 64], 1.0)
            ctxT = big.tile([128, NLP * N], bf16, name="ctxT")

            dmaq = [nc.sync, nc.gpsimd, nc.scalar]
            wkq_tiles = {}

            def dma_wkq(lp):
                wk_t = w_pool.tile([128, 8 * 128], bf16, name="wkp")
                nc.sync.dma_start(
                    out=wk_t[:], in_=wk_d[:, lp * 1024:(lp + 1) * 1024])
                wq_t = w_pool.tile([128, 8 * 128], bf16, name="wqp")
                nc.sync.dma_start(
                    out=wq_t[:], in_=wq_d[:, lp * 1024:(lp + 1) * 1024])
                wkq_tiles[lp] = (wk_t, wq_t)

            # ---- DMA schedule: wkq(0), XT stripes, WV, wkq(1) ----
            dma_wkq(0)
            for g in range(4):             # vertical stripe g: seq cols
                for cc in range(8):
                    dmaq[(g * 8 + cc) % 3].dma_start(
                        out=XT[:, cc * N + g * 512:cc * N + (g + 1) * 512],
                        in_=xT_d[:, cc * N + g * 512:cc * N + (g + 1) * 512])
            WV = wbig.tile([128, 8 * 512], bf16, name="wband")
            for cc in range(8):
                dmaq[cc % 3].dma_start(out=WV[:, cc * 512:(cc + 1) * 512],
                                       in_=wv_d[:, cc * 512:(cc + 1) * 512])
            dma_wkq(1)

            kts, qts = {}, {}

            def alloc_kq(lp):
                kts[lp] = kt_pool.tile([128, N], bf16, name="kT")
                qts[lp] = qt_pool.tile([128, N], bf16, name="qT")

            def kq_group(lp, which, g):
                wk_t, wq_t = wkq_tiles[lp]
                w, dst = (wk_t, kts[lp]) if which == "k" else (wq_t, qts[lp])
                ps = mm_ps.tile([128, 512], f32, name="ps")
                for cc in range(8):
                    nc.tensor.matmul(
                        ps[:], lhsT=w[:, cc * 128:(cc + 1) * 128],
                        rhs=XT[:, cc * N + g * 512:cc * N + (g + 1) * 512],
                        start=(cc == 0), stop=(cc == 7))
                nc.vector.tensor_copy(dst[:, g * 512:(g + 1) * 512], ps[:])

            def v_group(half, nsub):
                # half 0: local pairs 0-1 (head cols 0-255);
                # half 1: pairs 2-3 (cols 256-511)
                ps = mm_ps.tile([128, 512], f32, name="ps")
                for cc in range(8):
                    nc.tensor.matmul(
                        ps[:, :256],
                        lhsT=XT[:, cc * N + nsub * 128:cc * N + (nsub + 1) * 128],
                        rhs=WV[:, cc * 512 + half * 256:cc * 512 + half * 256 + 256],
                        start=(cc == 0), stop=(cc == 7))
                nc.vector.tensor_copy(
                    VA[:].rearrange("p (h c) -> p h c", h=8)
                       [:, half * 4:(half + 1) * 4, nsub * VW:nsub * VW + 64],
                    ps[:, :256].rearrange("p (h d) -> p h d", h=4))

            def ctx_mm(lp, hh, kc, pts, ctx_p):
                h = lp * 2 + hh
                for j in range(2):
                    nc.tensor.matmul(
                        ctx_p[:, j * 512:(j + 1) * 512],
                        lhsT=VA[:, (h * NKC + kc) * VW:(h * NKC + kc) * VW + 65],
                        rhs=pts[kc][:, j * 1024 + hh * 512:
                                    j * 1024 + hh * 512 + 512],
                        start=(kc == 0), stop=(kc == NKC - 1))

            def normalize(lp, qh, hh, ctx_p):
                r0 = hh * 64
                den = nrm_pool.tile([1, 1024], f32, name="den")
                nc.vector.tensor_copy(den[:], ctx_p[64:65, :])
                den_b = nrm_pool.tile([64, 1024], f32, name="den_b")
                nc.gpsimd.partition_broadcast(den_b[:], den[:])
                rec = nrm_pool.tile([64, 1024], f32, name="rec")
                nc.vector.reciprocal_approx_fast(out=rec[:], in_=den_b[:])
                nc.vector.tensor_tensor(
                    out=ctxT[r0:r0 + 64,
                             lp * N + qh * 1024:lp * N + qh * 1024 + 1024],
                    in0=ctx_p[:64, :], in1=rec[:],
                    op=mybir.AluOpType.mult)

            # ---- prologue ----
            alloc_kq(0)
            kq_group(0, "q", 0)
            kq_group(0, "k", 0)

            # ---- main loop: 8 blocks (lp, qh) ----
            for blk in range(8):
                lp, qh = divmod(blk, 2)
                if qh == 0:
                    if lp + 2 <= NLP - 1:
                        dma_wkq(lp + 2)
                    if lp + 1 <= NLP - 1:
                        alloc_kq(lp + 1)

                # fillers: one per kc slot, placed between the j0/j1 units
                slot_fill = []
                tail_fill = []
                pre_ctx_fill = []
                defer_ctx = False
                if blk == 0:
                    defer_ctx = True
                    slot_fill = [
                        lambda: kq_group(0, "q", 1),
                        lambda: kq_group(0, "k", 1),
                        lambda: kq_group(0, "k", 2),
                        lambda: kq_group(0, "k", 3),
                        lambda: kq_group(0, "q", 2),
                        lambda: kq_group(0, "q", 3),
                    ] + [(lambda ns=ns: v_group(0, ns)) for ns in range(10)]
                    pre_ctx_fill = [(lambda ns=ns: v_group(0, ns))
                                    for ns in range(10, 16)]
                elif blk == 1:
                    slot_fill = [(lambda g=g: kq_group(1, "k", g))
                                 for g in range(4)] + \
                                [(lambda g=g: kq_group(1, "q", g))
                                 for g in range(4)]
                elif blk in (2, 3):
                    # V for pairs 2-3 + K/Q for lp+1
                    lo = 8 * (blk - 2)
                    slot_fill = [(lambda ns=ns: v_group(1, ns))
                                 for ns in range(lo, lo + 8)]
                    tail_fill = [(lambda g=g: kq_group(2, "k", g))
                                 for g in range(2 * (blk - 2), 2 * (blk - 2) + 2)] + \
                                [(lambda g=g: kq_group(2, "q", g))
                                 for g in range(2 * (blk - 2), 2 * (blk - 2) + 2)]
                elif blk in (4, 5):
                    tail_fill = [(lambda g=g: kq_group(3, "k", g))
                                 for g in range(2 * (blk - 4), 2 * (blk - 4) + 2)] + \
                                [(lambda g=g: kq_group(3, "q", g))
                                 for g in range(2 * (blk - 4), 2 * (blk - 4) + 2)]

                kT, qT = kts[lp], qts[lp]
                pts = []
                ctx_e = None
                for kc in range(NKC):
                    pt = pt_pool.tile([128, 2048], bf16, name="pt")
                    for j in range(2):
                        sp = sc_ps.tile([128, 1024], f32, name="sc")
                        for hh in range(2):
                            r0 = hh * 64
                            for mh in range(2):
                                nc.tensor.matmul(
                                    sp[mh * 64:(mh + 1) * 64,
                                       hh * 512:(hh + 1) * 512],
                                    lhsT=kT[r0:r0 + 64,
                                            kc * 128 + mh * 64:
                                            kc * 128 + mh * 64 + 64],
                                    rhs=qT[r0:r0 + 64,
                                           qh * 1024 + j * 512:
                                           qh * 1024 + (j + 1) * 512],
                                    start=True, stop=True)
                        nc.scalar.activation(
                            pt[:, j * 1024:(j + 1) * 1024], sp[:],
                            mybir.ActivationFunctionType.Exp)
                        if j == 0 and kc < len(slot_fill):
                            slot_fill[kc]()
                    pts.append(pt)
                    if not defer_ctx:
                        if kc == 0:
                            ctx_e = ctx_ps.tile([VW, 1024], f32, name="ctx_p")
                        else:
                            ctx_mm(lp, 0, kc - 1, pts, ctx_e)
                for f in pre_ctx_fill:
                    f()
                if defer_ctx:
                    ctx_e = ctx_ps.tile([VW, 1024], f32, name="ctx_p")
                    for kc in range(1, NKC):
                        ctx_mm(lp, 0, kc - 1, pts, ctx_e)
                ctx_mm(lp, 0, NKC - 1, pts, ctx_e)
                normalize(lp, qh, 0, ctx_e)
                ctx_o = ctx_ps.tile([VW, 1024], f32, name="ctx_p")
                nf = 0
                for kc in range(NKC):
                    ctx_mm(lp, 1, kc, pts, ctx_o)
                    if kc % 3 == 2 and nf < len(tail_fill):
                        tail_fill[nf]()
                        nf += 1
                while nf < len(tail_fill):
                    tail_fill[nf]()
                    nf += 1
                normalize(lp, qh, 1, ctx_o)

            # ---- partial projection (host sums core pairs + bias) ----
            WO = wbig.tile([128, 4 * 1024], bf16, name="wband")
            for cc in range(4):
                dmaq[cc % 3].dma_start(out=WO[:, cc * 1024:(cc + 1) * 1024],
                                       in_=wo_d[:, cc * 1024:(cc + 1) * 1024])
            for nt in range(N // 128):
                for ch in range(2):
                    ps = mm_ps.tile([128, 512], f32, name="ps")
                    for cc in range(4):
                        nc.tensor.matmul(
                            ps[:],
                            lhsT=ctxT[:, cc * N + nt * 128:
                                      cc * N + nt * 128 + 128],
                            rhs=WO[:, cc * 1024 + ch * 512:
                                   cc * 1024 + ch * 512 + 512],
                            start=(cc == 0), stop=(cc == 3))
                    ot = out_pool.tile([128, 512], f32)
                    nc.vector.tensor_copy(ot[:], ps[:])
                    (nc.gpsimd, nc.sync)[(nt * 2 + ch) % 2].dma_start(
                        out=out_d[nt * 128:(nt + 1) * 128,
                                  ch * 512:(ch + 1) * 512],
                        in_=ot[:])

    nc.compile()
    return nc


def kernel(x, Wq, Wk, Wv, Wo, bo, _trace=False):
    from concourse.bass_utils import run_bass_kernel_spmd
    import ml_dtypes

    if "nc" not in _cache:
        _cache["nc"] = _build()
    nc = _cache["nc"]
    bf = ml_dtypes.bfloat16

    x = np.asarray(x, dtype=np.float32)
    Wq = np.asarray(Wq, dtype=np.float32) * SCALE
    Wk = np.asarray(Wk, dtype=np.float32)
    Wv = np.asarray(Wv, dtype=np.float32)
    Wo = np.asarray(Wo, dtype=np.float32)
    bo = np.asarray(bo, dtype=np.float32)

    def _pair_chunked(w, hg):
        # [C, C] -> [128, (lp, cc, 128)] for local pairs of head-group hg
        a = (w.astype(bf).reshape(8, 128, 8, 128)  # [cc, p, hp, d]
             [:, :, 4 * hg:4 * hg + 4, :])
        return np.ascontiguousarray(
            a.transpose(1, 2, 0, 3).reshape(128, NLP * 1024))

    def _wv_chunked(w, hg):
        # [C, C] -> [128, (cc, 512)]: local heads' columns
        a = w[:, 512 * hg:512 * hg + 512].astype(bf).reshape(8, 128, 512)
        return np.ascontiguousarray(a.transpose(1, 0, 2).reshape(128, 8 * 512))

    def _wo_chunked(w, hg):
        # rows [512*hg, 512*hg+512) of Wo -> [128, (cc, 1024)]
        a = w[512 * hg:512 * hg + 512, :].astype(bf).reshape(4, 128, 1024)
        return np.ascontiguousarray(a.transpose(1, 0, 2).reshape(128, 4 * 1024))

    wq = [_pair_chunked(Wq, hg) for hg in range(2)]
    wk = [_pair_chunked(Wk, hg) for hg in range(2)]
    wv = [_wv_chunked(Wv, hg) for hg in range(2)]
    wo = [_wo_chunked(Wo, hg) for hg in range(2)]

    xts = []
    for b in range(B):
        xT = x[b].T.astype(bf)
        xts.append(np.ascontiguousarray(
            xT.reshape(8, 128, N).transpose(1, 0, 2).reshape(128, 8 * N)))

    in_maps = []
    for c in range(NCORES):
        b, hg = divmod(c, 2)
        in_maps.append({"xT": xts[b], "wq": wq[hg], "wk": wk[hg],
                        "wv": wv[hg], "wo": wo[hg]})

    res = run_bass_kernel_spmd(nc, in_maps, core_ids=list(range(NCORES)),
                               trace=_trace, trace_cores=[0] if _trace else None)
    out = np.empty((B, N, C), dtype=np.float32)
    for b in range(B):
        out[b] = res.results[2 * b]["out"] + res.results[2 * b + 1]["out"] + bo
    if _trace:
        _cache["last_trace"] = res
    return out
